# revision 17
# baseline (speedup 1.0000x reference)
"""Trainium2 Bass kernel for nn_CapsuleLayer (dynamic routing).

Reference computation (B=128, I=1152, P=8, J=10, D=16):
    inputs_hat[b,i,j,d] = sum_p W[i,j,d,p] * inputs[b,i,p]
    b_logits = 0
    3x routing:
        c = softmax_j(b_logits)
        s[b,j,d] = sum_i c[b,i,j] * inputs_hat[b,i,j,d]
        outputs = squash(s)
        b_logits += sum_d inputs_hat[b,i,j,d] * outputs[b,j,d]   (iters 0,1)

Distribution: i-sharded across 8 cores (144 i's per core), full batch B=128
lives in the 128 SBUF partitions on every core.  The only cross-core traffic
is an 80KB AllReduce of the s-partials once per routing iteration.

Per-core layout: everything is [b=128 partitions, free], inputs_hat stored
bf16 as [128, IL, (d,j)=160].  PE materializes inputs_hat via per-i matmuls
(K=8) and computes iteration-0's s directly from a K=(i,p) matmul whose
AllReduce overlaps the inputs_hat phase.  DVE runs the bf16 2x elementwise
muls and halving-tree reductions; ACT handles exp and shares PSUM copies.
"""

import os
import sys
import functools

import numpy as np

if "/opt/trn_rl_repo" not in sys.path:
    sys.path.insert(0, "/opt/trn_rl_repo")

B = 128
I_FULL = 1152
P_DIM = 8
J = 10
D = 16
JD = D * J  # 160, flattened (d, j): col = d*J + j
NCORES = 8
ROUTINGS = 3
EPS = 1e-7

# lowering knobs (tuned for what actually runs on hardware)
N_HOMES = int(os.environ.get("K_N_HOMES", "3"))   # 32-aligned stationary homes
PSUM_CHUNK = int(os.environ.get("K_PSUM_CHUNK", "1"))  # i's per PSUM bank
K_STOP = int(os.environ.get("K_STOP", "0"))  # debug: truncate after stage N
BF16_OPS = os.environ.get("K_BF16_OPS", "0") == "1"  # bf16 matmul operands


def build(n_cores, IL, repeat=1):
    """Trace + compile the SPMD Bass program (one program, all cores)."""
    import concourse.bacc as bacc
    import concourse.bass as bass
    import concourse.mybir as mybir
    import concourse.tile as tile
    from concourse.masks import make_identity

    F32 = mybir.dt.float32
    BF16 = mybir.dt.bfloat16
    AF = mybir.ActivationFunctionType
    OP = mybir.AluOpType
    AX = mybir.AxisListType

    assert IL % 16 == 0
    G = IL // 16  # number of 128-row (16 i x 8 p) k-tiles

    nc = bacc.Bacc(
        "TRN2", target_bir_lowering=False, debug=False, num_devices=n_cores
    )
    x_d = nc.dram_tensor("x", [B, IL, P_DIM], F32, kind="ExternalInput").ap()
    w_d = nc.dram_tensor("w", [IL, J, D, P_DIM], F32, kind="ExternalInput").ap()
    out_d = nc.dram_tensor("out", [B, J, D], F32, kind="ExternalOutput").ap()

    with tile.TileContext(nc, num_cores=n_cores) as tc:
        for rep in range(repeat):
            _trace(tc, nc, x_d, w_d, out_d, n_cores, IL, G, F32, BF16, AF,
                   OP, AX, bass, mybir, make_identity, rep)

    nc.compile()
    return nc


def _trace(tc, nc, x_d, w_d, out_d, n_cores, IL, G, F32, BF16, AF, OP, AX,
           bass, mybir, make_identity, rep=0):
    import contextlib

    ctx = contextlib.ExitStack()
    with ctx:
        singles = ctx.enter_context(
            tc.tile_pool(name=f"singles{rep}", bufs=1))
        stage = ctx.enter_context(tc.tile_pool(name=f"stage{rep}", bufs=3))
        big = ctx.enter_context(tc.tile_pool(name=f"big{rep}", bufs=1))
        small = ctx.enter_context(tc.tile_pool(name=f"small{rep}", bufs=3))
        psT = ctx.enter_context(
            tc.tile_pool(name=f"psT{rep}", bufs=2, space="PSUM"))
        psS = ctx.enter_context(
            tc.tile_pool(name=f"psS{rep}", bufs=1, space="PSUM"))
        psIH = ctx.enter_context(
            tc.tile_pool(name=f"psIH{rep}", bufs=4, space="PSUM"))
        dram = ctx.enter_context(
            tc.tile_pool(name=f"dram{rep}", bufs=1, space="DRAM"))

        # ---- constants -------------------------------------------------
        ident = singles.tile([128, 128], F32)
        make_identity(nc, ident[:])
        dummy = singles.tile([128, 1], F32)
        nc.vector.memset(dummy[:], 0.0)
        eps_t = singles.tile([128, 1], F32)
        nc.vector.memset(eps_t[:], EPS)
        # preload ACT tables (Exp / Log) before the hot loop
        nc.scalar.activation(dummy[:], dummy[:], AF.Exp)

        # ---- load inputs, build transposed operands --------------------
        # x_nat: [b, (i p)] fp32
        x_nat = big.tile([128, IL * P_DIM], F32)
        nc.sync.dma_start(out=x_nat[:], in_=x_d.rearrange("b i p -> b (i p)"))

        OPDT = BF16 if BF16_OPS else F32
        # xT[k, g, b]: k-tile g holds rows (i_loc*8+p) for i in [16g,16g+16)
        xT = big.tile([128, G, 128], OPDT)
        for g in range(G):
            pst = psT.tile([128, 128], F32, tag="pst")
            nc.tensor.transpose(
                pst[:], x_nat[:, g * 128:(g + 1) * 128], ident[:])
            nc.vector.tensor_copy(xT[:, g, :], pst[:])

        # W2[k, g, (d j)]: same k-row ordering, free dim is (d,j) = d*J + j
        W2 = big.tile([128, G, JD], OPDT)
        for g in range(G):
            # staging: Wn rows = (d,j) (2 tiles: 0..127 and 128..159),
            # cols = (i_loc, p)
            wna = stage.tile([128, 128], F32, tag="wna")
            wnb = stage.tile([32, 128], F32, tag="wnb")
            i0 = 16 * g
            for d in range(D):
                base = d * J  # global row of (d, j=0)
                segs = []  # (dst_tile, dst_row0, j0, j1)
                if base < 128:
                    segs.append((wna, base, 0, min(J, 128 - base)))
                if base + J > 128:
                    j0 = max(0, 128 - base)
                    segs.append((wnb, base + j0 - 128, j0, J))
                for (tt, tr, js, je) in segs:
                    sl = w_d[i0:i0 + 16, js:je, d:d + 1, :]
                    nc.sync.dma_start(
                        out=tt[tr:tr + (je - js), :].rearrange(
                            "r (i p) -> r i p", p=P_DIM),
                        in_=sl.rearrange("i j d p -> (j d) i p"),
                    )
            psa = psT.tile([128, 128], F32, tag="pst")
            nc.tensor.transpose(psa[:], wna[:], ident[:])
            nc.vector.tensor_copy(W2[:, g, 0:128], psa[:])
            psb = psT.tile([128, 32], F32, tag="pst")
            nc.tensor.transpose(psb[:], wnb[:], ident[0:32, 0:32])
            nc.vector.tensor_copy(W2[:, g, 128:JD], psb[:])

        # ---- spread copies: each i's 8 (p-)rows at a 32-aligned base ----
        # PE stationary slices must start at partition 0/32/64(/96), so the
        # per-i K=8 matmuls read from these "home" tiles instead of the
        # dense k-tiles.  i -> (home q = i%N_HOMES at base 32q, slot i//N).
        n_slot = (IL + N_HOMES - 1) // N_HOMES
        xt8 = big.tile([128, n_slot, 128], OPDT)
        w28 = big.tile([128, n_slot, JD], OPDT)
        for i in range(IL):
            g, r = i // 16, i % 16
            q, slot = i % N_HOMES, i // N_HOMES
            nc.sync.dma_start(out=xt8[32 * q:32 * q + 8, slot, :],
                              in_=xT[8 * r:8 * r + 8, g, :])
            nc.sync.dma_start(out=w28[32 * q:32 * q + 8, slot, :],
                              in_=W2[8 * r:8 * r + 8, g, :])

        # ---- iteration-0 s directly from PE (c == 1/J), AllReduce now --
        # s0T[(d j), b] = sum_{(i,p)} W2[k, dj] * xT[k, b]
        ps_a = psS.tile([128, 128], F32, tag="s0a")
        ps_b = psS.tile([32, 128], F32, tag="s0b")
        for g in range(G):
            nc.tensor.matmul(ps_a[:], W2[:, g, 0:128], xT[:, g, :],
                             start=(g == 0), stop=(g == G - 1))
        for g in range(G):
            nc.tensor.matmul(ps_b[:], W2[:, g, 128:JD], xT[:, g, :],
                             start=(g == 0), stop=(g == G - 1))
        # scale by 1/J while copying out of PSUM, then transpose to [b, dj]
        s0T_a = stage.tile([128, 128], F32, tag="s0Ta")
        s0T_b = stage.tile([32, 128], F32, tag="s0Tb")
        nc.scalar.mul(s0T_a[:], ps_a[:], 1.0 / J)
        nc.scalar.mul(s0T_b[:], ps_b[:], 1.0 / J)
        s0p = small.tile([128, JD], F32, tag="spart")
        pst = psT.tile([128, 128], F32, tag="pst")
        nc.tensor.transpose(pst[:], s0T_a[:], ident[:])
        nc.vector.tensor_copy(s0p[:, 0:128], pst[:])
        pstb2 = psT.tile([128, 32], F32, tag="pst")
        nc.tensor.transpose(pstb2[:], s0T_b[:], ident[0:32, 0:32])
        nc.vector.tensor_copy(s0p[:, 128:JD], pstb2[:])

        def all_reduce(s_part, tag):
            cc_in = dram.tile([B, JD], F32, name=f"ccin_{tag}")
            cc_out = dram.tile([B, JD], F32, name=f"ccout_{tag}",
                               addr_space="Shared")
            nc.gpsimd.dma_start(out=cc_in[:], in_=s_part[:])
            if n_cores > 1:
                nc.gpsimd.collective_compute(
                    "AllReduce",
                    OP.add,
                    replica_groups=[list(range(n_cores))],
                    ins=[cc_in[:].opt()],
                    outs=[cc_out[:].opt()],
                )
            else:
                nc.gpsimd.dma_start(out=cc_out[:], in_=cc_in[:])
            s_glob = small.tile([128, JD], F32, tag="sglob")
            nc.gpsimd.dma_start(out=s_glob[:], in_=cc_out[:])
            return s_glob

        s0g = all_reduce(s0p, "s0")  # overlaps the IH phase below

        # ---- materialize inputs_hat: IH[b, i, (d j)] bf16 --------------
        IH = big.tile([128, IL, JD], BF16)
        CH = PSUM_CHUNK
        n3 = (IL + CH - 1) // CH
        for k3 in range(n3):
            csz = min(CH, IL - CH * k3)
            ps = psIH.tile([128, CH, JD], F32, tag="ih")
            for t in range(csz):
                i = CH * k3 + t
                q, slot = i % N_HOMES, i // N_HOMES
                nc.tensor.matmul(ps[:, t, :],
                                 xt8[32 * q:32 * q + 8, slot, :],
                                 w28[32 * q:32 * q + 8, slot, :],
                                 start=True, stop=True,
                                 tile_position=(32 * q, 0))
            dst = IH[:, CH * k3:CH * k3 + csz, :]
            if k3 % 2 == 0:
                nc.vector.tensor_copy(dst, ps[:, 0:csz, :])
            else:
                nc.scalar.copy(dst, ps[:, 0:csz, :])

        # ---- routing helpers -------------------------------------------
        XB = big.tile([128, IL, JD], BF16)  # scratch for muls + trees
        L = big.tile([128, IL, J], F32)     # routing logits

        def squash(s_glob, want_bf16):
            """squash along d of s_glob[128,(d j)] -> (f32, bf16|None)."""
            sq = small.tile([128, JD], F32, tag="sq")
            nc.vector.tensor_mul(sq[:], s_glob[:], s_glob[:])
            s2 = small.tile([128, J], F32, tag="s2")
            nc.vector.reduce_sum(
                s2[:], sq.rearrange("b (d j) -> b j d", d=D, j=J), axis=AX.X)
            # t = sqrt(s2 + eps) = exp(0.5 * ln(s2 + eps))
            lt = small.tile([128, J], F32, tag="lt")
            nc.scalar.activation(lt[:], s2[:], AF.Ln, bias=eps_t[:])
            rt = small.tile([128, J], F32, tag="rt")
            nc.scalar.activation(rt[:], lt[:], AF.Exp, scale=-0.5)  # 1/sqrt
            u = small.tile([128, J], F32, tag="u")
            nc.vector.tensor_scalar_add(u[:], s2[:], 1.0)
            ru = small.tile([128, J], F32, tag="ru")
            nc.vector.reciprocal(ru[:], u[:])
            sc = small.tile([128, J], F32, tag="sc")
            nc.vector.tensor_mul(sc[:], s2[:], ru[:])
            nc.vector.tensor_mul(sc[:], sc[:], rt[:])
            o_f = small.tile([128, JD], F32, tag="of")
            sc_b = sc[:].unsqueeze(1).broadcast_to([128, D, J])
            nc.vector.tensor_tensor(
                o_f.rearrange("b (d j) -> b d j", d=D, j=J),
                s_glob.rearrange("b (d j) -> b d j", d=D, j=J),
                sc_b, op=OP.mult)
            o_b = None
            if want_bf16:
                o_b = small.tile([128, JD], BF16, tag="ob")
                nc.vector.tensor_copy(o_b[:], o_f[:])
            return o_f, o_b

        def agreement(o_b, first):
            """XB = IH * o_b (bcast over i); tree-reduce d; into L."""
            nc.vector.tensor_tensor(
                XB[:], IH[:],
                o_b[:].unsqueeze(1).broadcast_to([128, IL, JD]), op=OP.mult)
            w = JD
            while w > 2 * J:
                h = w // 2
                nc.vector.tensor_tensor(
                    XB[:, :, 0:h], XB[:, :, 0:h], XB[:, :, h:w], op=OP.add)
                w = h
            # final level: w == 2J -> write/accumulate into logits
            if first:
                nc.vector.tensor_tensor(
                    L[:], XB[:, :, 0:J], XB[:, :, J:2 * J], op=OP.add)
            else:
                a1 = big.tile([128, IL, J], F32, tag="a1")
                nc.vector.tensor_tensor(
                    a1[:], XB[:, :, 0:J], XB[:, :, J:2 * J], op=OP.add)
                nc.vector.tensor_tensor(L[:], L[:], a1[:], op=OP.add)

        def softmax():
            """c = softmax_j(L) -> bf16 [128, IL, J]."""
            E = big.tile([128, IL, J], F32, tag="E")
            nc.scalar.activation(E[:], L[:], AF.Exp)
            Z = small.tile([128, IL], F32, tag="Z")
            nc.vector.reduce_sum(Z[:], E[:], axis=AX.X)
            R = small.tile([128, IL], F32, tag="R")
            nc.vector.reciprocal(R[:], Z[:])
            Cb = big.tile([128, IL, J], BF16, tag="Cb")
            nc.vector.tensor_tensor(
                Cb[:], E[:], R[:].unsqueeze(2).broadcast_to([128, IL, J]),
                op=OP.mult)
            return Cb

        def weighted_sum(Cb, tag):
            """XB = IH * c (bcast over d); tree-reduce i -> s_part."""
            nc.vector.tensor_tensor(
                XB.rearrange("b i (d j) -> b i d j", d=D, j=J),
                IH.rearrange("b i (d j) -> b i d j", d=D, j=J),
                Cb[:].unsqueeze(2).broadcast_to([128, IL, D, J]),
                op=OP.mult)
            n = IL
            while n > 1:
                h = n // 2
                nc.vector.tensor_tensor(
                    XB[:, 0:h, :], XB[:, 0:h, :], XB[:, h:2 * h, :],
                    op=OP.add)
                if n % 2:
                    nc.vector.tensor_tensor(
                        XB[:, 0:1, :], XB[:, 0:1, :], XB[:, n - 1:n, :],
                        op=OP.add)
                n = h
            s_part = small.tile([128, JD], F32, tag="spart")
            nc.vector.tensor_copy(s_part[:], XB[:, 0, :])
            return s_part

        # ---- routing loop ----------------------------------------------
        # iter 0
        o_f, o_b = squash(s0g, want_bf16=True)
        agreement(o_b, first=True)
        Cb = softmax()
        # iter 1
        s1p = weighted_sum(Cb, "s1")
        s1g = all_reduce(s1p, "s1")
        o_f, o_b = squash(s1g, want_bf16=True)
        agreement(o_b, first=False)
        Cb = softmax()
        # iter 2
        s2p = weighted_sum(Cb, "s2")
        s2g = all_reduce(s2p, "s2")
        o_f, _ = squash(s2g, want_bf16=False)

        # reorder (d,j) -> (j,d) and store
        OUTJD = small.tile([128, J, D], F32, tag="outjd")
        nc.vector.tensor_copy(
            OUTJD[:], o_f.rearrange("b (d j) -> b j d", d=D, j=J))
        nc.sync.dma_start(out=out_d[:], in_=OUTJD[:])


@functools.lru_cache(maxsize=None)
def _get_nc():
    return build(NCORES, I_FULL // NCORES)


def kernel(inputs, W):
    """Full-input entry point: inputs [128,1152,8] f32, W [1,1152,10,16,8]."""
    from concourse.bass_utils import run_bass_kernel_spmd

    inputs = np.ascontiguousarray(np.asarray(inputs), dtype=np.float32)
    W0 = np.ascontiguousarray(np.asarray(W)[0], dtype=np.float32)
    IL = I_FULL // NCORES
    nc = _get_nc()
    in_maps = [
        {
            "x": np.ascontiguousarray(inputs[:, c * IL:(c + 1) * IL, :]),
            "w": np.ascontiguousarray(W0[c * IL:(c + 1) * IL]),
        }
        for c in range(NCORES)
    ]
    res = run_bass_kernel_spmd(nc, in_maps, core_ids=list(range(NCORES)))
    return np.asarray(res.results[0]["out"], dtype=np.float32)


if __name__ == "__main__":
    nc = build(1, 16)
    print("built OK")


# revision 23
# speedup vs baseline: 202.6970x; 202.6970x over previous
"""Trainium2 Bass kernel for nn_CapsuleLayer (dynamic routing).

Reference computation (B=128, I=1152, P=8, J=10, D=16):
    inputs_hat[b,i,j,d] = sum_p W[i,j,d,p] * inputs[b,i,p]
    b_logits = 0
    3x routing:
        c = softmax_j(b_logits)
        s[b,j,d] = sum_i c[b,i,j] * inputs_hat[b,i,j,d]
        outputs = squash(s)
        b_logits += sum_d inputs_hat[b,i,j,d] * outputs[b,j,d]   (iters 0,1)

Distribution: i-sharded across 8 cores (144 i's per core), full batch B=128
lives in the 128 SBUF partitions on every core.  The only cross-core traffic
is an 80KB AllReduce of the s-partials once per routing iteration.

Per-core layout: everything is [b=128 partitions, free], inputs_hat stored
bf16 as [128, IL, (d,j)=160].  PE materializes inputs_hat via per-i matmuls
(K=8) and computes iteration-0's s directly from a K=(i,p) matmul whose
AllReduce overlaps the inputs_hat phase.  DVE runs the bf16 2x elementwise
muls and halving-tree reductions; ACT handles exp and shares PSUM copies.
"""

import os
import sys
import functools

import numpy as np

if "/opt/trn_rl_repo" not in sys.path:
    sys.path.insert(0, "/opt/trn_rl_repo")

B = 128
I_FULL = 1152
P_DIM = 8
J = 10
D = 16
JD = D * J  # 160, flattened (d, j): col = d*J + j
NCORES = 8
ROUTINGS = 3
EPS = 1e-7

# lowering knobs (tuned for what actually runs on hardware)
N_HOMES = int(os.environ.get("K_N_HOMES", "3"))   # 32-aligned stationary homes
PSUM_CHUNK = int(os.environ.get("K_PSUM_CHUNK", "1"))  # i's per PSUM bank
K_STOP = int(os.environ.get("K_STOP", "0"))  # debug: truncate after stage N
GPS_FRAC = float(os.environ.get("K_GPS_FRAC", "0"))  # i-frac of muls on gpsimd
BF16_OPS = os.environ.get("K_BF16_OPS", "0") == "1"  # bf16 matmul operands


def build(n_cores, IL, repeat=1):
    """Trace + compile the SPMD Bass program (one program, all cores)."""
    import concourse.bacc as bacc
    import concourse.bass as bass
    import concourse.mybir as mybir
    import concourse.tile as tile
    from concourse.masks import make_identity

    F32 = mybir.dt.float32
    BF16 = mybir.dt.bfloat16
    AF = mybir.ActivationFunctionType
    OP = mybir.AluOpType
    AX = mybir.AxisListType

    assert IL % 16 == 0
    G = IL // 16  # number of 128-row (16 i x 8 p) k-tiles

    nc = bacc.Bacc(
        "TRN2", target_bir_lowering=False, debug=False, num_devices=n_cores
    )
    x_d = nc.dram_tensor("x", [B, IL, P_DIM], F32, kind="ExternalInput").ap()
    w_d = nc.dram_tensor("w", [IL, J, D, P_DIM], F32, kind="ExternalInput").ap()
    out_d = nc.dram_tensor("out", [B, J, D], F32, kind="ExternalOutput").ap()

    with tile.TileContext(nc, num_cores=n_cores) as tc:
        for rep in range(repeat):
            _trace(tc, nc, x_d, w_d, out_d, n_cores, IL, G, F32, BF16, AF,
                   OP, AX, bass, mybir, make_identity, rep)

    nc.compile()
    return nc


def _trace(tc, nc, x_d, w_d, out_d, n_cores, IL, G, F32, BF16, AF, OP, AX,
           bass, mybir, make_identity, rep=0):
    import contextlib

    ctx = contextlib.ExitStack()
    with ctx:
        singles = ctx.enter_context(
            tc.tile_pool(name=f"singles{rep}", bufs=1))
        stage = ctx.enter_context(tc.tile_pool(name=f"stage{rep}", bufs=3))
        big = ctx.enter_context(tc.tile_pool(name=f"big{rep}", bufs=1))
        small = ctx.enter_context(tc.tile_pool(name=f"small{rep}", bufs=3))
        psT = ctx.enter_context(
            tc.tile_pool(name=f"psT{rep}", bufs=2, space="PSUM"))
        psS = ctx.enter_context(
            tc.tile_pool(name=f"psS{rep}", bufs=1, space="PSUM"))
        psIH = ctx.enter_context(
            tc.tile_pool(name=f"psIH{rep}", bufs=4, space="PSUM"))
        dram = ctx.enter_context(
            tc.tile_pool(name=f"dram{rep}", bufs=1, space="DRAM"))

        # ---- constants -------------------------------------------------
        ident = singles.tile([128, 128], F32)
        make_identity(nc, ident[:])
        dummy = singles.tile([128, 1], F32)
        nc.vector.memset(dummy[:], 0.0)
        eps_t = singles.tile([128, 1], F32)
        nc.vector.memset(eps_t[:], EPS)
        # preload ACT tables (Exp / Log) before the hot loop
        nc.scalar.activation(dummy[:], dummy[:], AF.Exp)

        # ---- load inputs, build transposed operands --------------------
        # x_nat: [b, (i p)] fp32
        x_nat = big.tile([128, IL * P_DIM], F32)
        nc.sync.dma_start(out=x_nat[:], in_=x_d.rearrange("b i p -> b (i p)"))

        OPDT = BF16 if BF16_OPS else F32
        # xT[k, g, b]: k-tile g holds rows (i_loc*8+p) for i in [16g,16g+16)
        xT = big.tile([128, G, 128], OPDT)
        for g in range(G):
            pst = psT.tile([128, 128], F32, tag="pst")
            nc.tensor.transpose(
                pst[:], x_nat[:, g * 128:(g + 1) * 128], ident[:])
            nc.vector.tensor_copy(xT[:, g, :], pst[:])

        # W2[k, g, (d j)]: same k-row ordering, free dim is (d,j) = d*J + j.
        # Staging uses (j,d) rows so each j is ONE contiguous-ish DMA (16
        # rows); the PSUM->SBUF copy permutes cols back to (d,j).  DMA
        # triggering alternates sync/tensor queues to parallelize the
        # per-dma_start sequencer cost.
        W2 = big.tile([128, G, JD], OPDT)
        dma_engs = [nc.sync, nc.scalar]
        for g in range(G):
            wna = stage.tile([128, 128], F32, tag="wna")  # rows j*16+d, j<8
            wnb = stage.tile([32, 128], F32, tag="wnb")   # rows (j-8)*16+d
            i0 = 16 * g
            for j in range(J):
                tt, tr = (wna, j * 16) if j < 8 else (wnb, (j - 8) * 16)
                sl = w_d[i0:i0 + 16, j:j + 1, :, :]
                dma_engs[(g * J + j) % 2].dma_start(
                    out=tt[tr:tr + 16, :].rearrange(
                        "r (i p) -> r i p", p=P_DIM),
                    in_=sl.rearrange("i j d p -> (j d) i p"),
                )
            W2g = W2[:, g, :].rearrange("k (d j) -> k d j", d=D, j=J)
            psa = psT.tile([128, 128], F32, tag="pst")
            nc.tensor.transpose(psa[:], wna[:], ident[:])
            nc.vector.tensor_copy(
                W2g[:, :, 0:8],
                psa[:].rearrange("k (j d) -> k j d", j=8, d=D).transpose(
                    [0, 2, 1]))
            psb = psT.tile([128, 32], F32, tag="pst")
            nc.tensor.transpose(psb[:], wnb[:], ident[0:32, 0:32])
            nc.vector.tensor_copy(
                W2g[:, :, 8:10],
                psb[:].rearrange("k (j d) -> k j d", j=2, d=D).transpose(
                    [0, 2, 1]))

        # ---- block-diagonal weight tiles for the inputs_hat matmuls -----
        # K=32 slices of the dense k-tiles are legal stationary bases
        # (0/32/64/96 with explicit tile_position).  Each 32-row group
        # holds 4 i's; the moving operand is a [32, 640] block-diagonal
        # expansion of W2 so the 4 i's don't mix.  Built with
        # partition-preserving on-chip copies (rows 32a+8t == 8*i_loc),
        # no DMA involved.
        w28bd = big.tile([128, G, 4 * JD], F32)
        nc.gpsimd.memset(w28bd[:], 0.0)
        idx = 0
        for g in range(G):
            for a in range(4):
                for t in range(4):
                    r0 = 32 * a + 8 * t
                    dst = w28bd[r0:r0 + 8, g, JD * t:JD * (t + 1)]
                    src = W2[r0:r0 + 8, g, :]
                    if t == 0:
                        # 32-aligned start partition: engine copy is legal
                        nc.vector.tensor_copy(dst, src)
                    else:
                        # sub-32 partition starts: only DMA can address
                        dma_engs[idx % 2].dma_start(out=dst, in_=src)
                        idx += 1

        # ---- iteration-0 s directly from PE (c == 1/J), AllReduce now --
        # s0T[(d j), b] = sum_{(i,p)} W2[k, dj] * xT[k, b]
        ps_a = psS.tile([128, 128], F32, tag="s0a")
        ps_b = psS.tile([32, 128], F32, tag="s0b")
        for g in range(G):
            nc.tensor.matmul(ps_a[:], W2[:, g, 0:128], xT[:, g, :],
                             start=(g == 0), stop=(g == G - 1))
        for g in range(G):
            nc.tensor.matmul(ps_b[:], W2[:, g, 128:JD], xT[:, g, :],
                             start=(g == 0), stop=(g == G - 1))
        # scale by 1/J while copying out of PSUM, then transpose to [b, dj]
        s0T_a = stage.tile([128, 128], F32, tag="s0Ta")
        s0T_b = stage.tile([32, 128], F32, tag="s0Tb")
        nc.scalar.mul(s0T_a[:], ps_a[:], 1.0 / J)
        nc.scalar.mul(s0T_b[:], ps_b[:], 1.0 / J)
        s0p = small.tile([128, JD], F32, tag="spart")
        pst = psT.tile([128, 128], F32, tag="pst")
        nc.tensor.transpose(pst[:], s0T_a[:], ident[:])
        nc.vector.tensor_copy(s0p[:, 0:128], pst[:])
        pstb2 = psT.tile([128, 32], F32, tag="pst")
        nc.tensor.transpose(pstb2[:], s0T_b[:], ident[0:32, 0:32])
        nc.vector.tensor_copy(s0p[:, 128:JD], pstb2[:])

        def all_reduce(s_part, tag):
            cc_in = dram.tile([B, JD], F32, name=f"ccin_{tag}")
            cc_out = dram.tile([B, JD], F32, name=f"ccout_{tag}",
                               addr_space="Shared")
            nc.gpsimd.dma_start(out=cc_in[:], in_=s_part[:])
            if n_cores > 1 and os.environ.get("K_NO_CC", "0") != "1":
                nc.gpsimd.collective_compute(
                    "AllReduce",
                    OP.add,
                    replica_groups=[list(range(n_cores))],
                    ins=[cc_in[:].opt()],
                    outs=[cc_out[:].opt()],
                )
            else:
                nc.gpsimd.dma_start(out=cc_out[:], in_=cc_in[:])
            s_glob = small.tile([128, JD], F32, tag="sglob")
            nc.gpsimd.dma_start(out=s_glob[:], in_=cc_out[:])
            return s_glob

        s0g = all_reduce(s0p, "s0")  # overlaps the IH phase below

        # ---- materialize inputs_hat: IH[b, i, (d j)] bf16 --------------
        IH = big.tile([128, IL, JD], BF16)
        kk = 0
        for g in range(G):
            for a in range(4):
                for h in range(2):
                    i0 = 16 * g + 4 * a + 2 * h
                    if i0 >= IL:
                        continue
                    ps = psIH.tile([128, 2 * JD], F32, tag="ih")
                    nc.tensor.matmul(
                        ps[:], xT[32 * a:32 * a + 32, g, :],
                        w28bd[32 * a:32 * a + 32, g,
                              2 * JD * h:2 * JD * (h + 1)],
                        start=True, stop=True, tile_position=(32 * a, 0))
                    dst = IH[:, i0:i0 + 2, :]
                    if kk % 3 == 0:
                        nc.vector.tensor_copy(dst, ps[:])
                    else:
                        nc.scalar.copy(dst, ps[:])
                    kk += 1

        # ---- routing helpers -------------------------------------------
        XB = big.tile([128, IL, JD], BF16)  # scratch for muls + trees
        L = big.tile([128, IL, J], F32)     # routing logits

        def squash(s_glob, want_bf16):
            """squash along d of s_glob[128,(d j)] -> (f32, bf16|None)."""
            sq = small.tile([128, JD], F32, tag="sq")
            nc.vector.tensor_mul(sq[:], s_glob[:], s_glob[:])
            s2 = small.tile([128, J], F32, tag="s2")
            nc.vector.reduce_sum(
                s2[:], sq.rearrange("b (d j) -> b j d", d=D, j=J), axis=AX.X)
            # t = sqrt(s2 + eps) = exp(0.5 * ln(s2 + eps))
            lt = small.tile([128, J], F32, tag="lt")
            nc.scalar.activation(lt[:], s2[:], AF.Ln, bias=eps_t[:])
            rt = small.tile([128, J], F32, tag="rt")
            nc.scalar.activation(rt[:], lt[:], AF.Exp, scale=-0.5)  # 1/sqrt
            u = small.tile([128, J], F32, tag="u")
            nc.vector.tensor_scalar_add(u[:], s2[:], 1.0)
            ru = small.tile([128, J], F32, tag="ru")
            nc.vector.reciprocal(ru[:], u[:])
            sc = small.tile([128, J], F32, tag="sc")
            nc.vector.tensor_mul(sc[:], s2[:], ru[:])
            nc.vector.tensor_mul(sc[:], sc[:], rt[:])
            o_f = small.tile([128, JD], F32, tag="of")
            sc_b = sc[:].unsqueeze(1).broadcast_to([128, D, J])
            nc.vector.tensor_tensor(
                o_f.rearrange("b (d j) -> b d j", d=D, j=J),
                s_glob.rearrange("b (d j) -> b d j", d=D, j=J),
                sc_b, op=OP.mult)
            o_b = None
            if want_bf16:
                o_b = small.tile([128, JD], BF16, tag="ob")
                nc.vector.tensor_copy(o_b[:], o_f[:])
            return o_f, o_b

        def agreement(o_b, first):
            """XB = IH * o_b (bcast over i); tree-reduce d; into L."""
            IS = IL - int(IL * GPS_FRAC) if GPS_FRAC > 0 else IL
            nc.vector.tensor_tensor(
                XB[:, 0:IS, :], IH[:, 0:IS, :],
                o_b[:].unsqueeze(1).broadcast_to([128, IS, JD]), op=OP.mult)
            if IS < IL:
                nc.gpsimd.tensor_tensor(
                    XB[:, IS:IL, :], IH[:, IS:IL, :],
                    o_b[:].unsqueeze(1).broadcast_to([128, IL - IS, JD]),
                    op=OP.mult)
            w = JD
            while w > 2 * J:
                h = w // 2
                nc.vector.tensor_tensor(
                    XB[:, :, 0:h], XB[:, :, 0:h], XB[:, :, h:w], op=OP.add)
                w = h
            # final level: w == 2J -> write/accumulate into logits
            if first:
                nc.vector.tensor_tensor(
                    L[:], XB[:, :, 0:J], XB[:, :, J:2 * J], op=OP.add)
            else:
                a1 = big.tile([128, IL, J], F32, tag="a1")
                nc.vector.tensor_tensor(
                    a1[:], XB[:, :, 0:J], XB[:, :, J:2 * J], op=OP.add)
                nc.vector.tensor_tensor(L[:], L[:], a1[:], op=OP.add)

        def softmax():
            """c = softmax_j(L) -> bf16 [128, IL, J]."""
            E = big.tile([128, IL, J], F32, tag="E")
            nc.scalar.activation(E[:], L[:], AF.Exp)
            Z = small.tile([128, IL], F32, tag="Z")
            nc.vector.reduce_sum(Z[:], E[:], axis=AX.X)
            R = small.tile([128, IL], F32, tag="R")
            nc.vector.reciprocal(R[:], Z[:])
            Cb = big.tile([128, IL, J], BF16, tag="Cb")
            nc.vector.tensor_tensor(
                Cb[:], E[:], R[:].unsqueeze(2).broadcast_to([128, IL, J]),
                op=OP.mult)
            return Cb

        def weighted_sum(Cb, tag):
            """XB = IH * c (bcast over d); tree-reduce i -> s_part."""
            XBv = XB.rearrange("b i (d j) -> b i d j", d=D, j=J)
            IHv = IH.rearrange("b i (d j) -> b i d j", d=D, j=J)
            Cbv = Cb[:].unsqueeze(2).broadcast_to([128, IL, D, J])
            IS = IL - int(IL * GPS_FRAC) if GPS_FRAC > 0 else IL
            nc.vector.tensor_tensor(
                XBv[:, 0:IS], IHv[:, 0:IS], Cbv[:, 0:IS], op=OP.mult)
            if IS < IL:
                nc.gpsimd.tensor_tensor(
                    XBv[:, IS:IL], IHv[:, IS:IL], Cbv[:, IS:IL], op=OP.mult)
            n = IL
            while n > 1:
                h = n // 2
                nc.vector.tensor_tensor(
                    XB[:, 0:h, :], XB[:, 0:h, :], XB[:, h:2 * h, :],
                    op=OP.add)
                if n % 2:
                    nc.vector.tensor_tensor(
                        XB[:, 0:1, :], XB[:, 0:1, :], XB[:, n - 1:n, :],
                        op=OP.add)
                n = h
            s_part = small.tile([128, JD], F32, tag="spart")
            nc.vector.tensor_copy(s_part[:], XB[:, 0, :])
            return s_part

        # ---- routing loop ----------------------------------------------
        # iter 0
        o_f, o_b = squash(s0g, want_bf16=True)
        agreement(o_b, first=True)
        Cb = softmax()
        # iter 1
        s1p = weighted_sum(Cb, "s1")
        s1g = all_reduce(s1p, "s1")
        o_f, o_b = squash(s1g, want_bf16=True)
        agreement(o_b, first=False)
        Cb = softmax()
        # iter 2
        s2p = weighted_sum(Cb, "s2")
        s2g = all_reduce(s2p, "s2")
        o_f, _ = squash(s2g, want_bf16=False)

        # reorder (d,j) -> (j,d) and store
        OUTJD = small.tile([128, J, D], F32, tag="outjd")
        nc.vector.tensor_copy(
            OUTJD[:], o_f.rearrange("b (d j) -> b j d", d=D, j=J))
        nc.sync.dma_start(out=out_d[:], in_=OUTJD[:])


@functools.lru_cache(maxsize=None)
def _get_nc():
    return build(NCORES, I_FULL // NCORES)


def kernel(inputs, W):
    """Full-input entry point: inputs [128,1152,8] f32, W [1,1152,10,16,8]."""
    from concourse.bass_utils import run_bass_kernel_spmd

    inputs = np.ascontiguousarray(np.asarray(inputs), dtype=np.float32)
    W0 = np.ascontiguousarray(np.asarray(W)[0], dtype=np.float32)
    IL = I_FULL // NCORES
    nc = _get_nc()
    in_maps = [
        {
            "x": np.ascontiguousarray(inputs[:, c * IL:(c + 1) * IL, :]),
            "w": np.ascontiguousarray(W0[c * IL:(c + 1) * IL]),
        }
        for c in range(NCORES)
    ]
    res = run_bass_kernel_spmd(nc, in_maps, core_ids=list(range(NCORES)))
    return np.asarray(res.results[0]["out"], dtype=np.float32)


if __name__ == "__main__":
    nc = build(1, 16)
    print("built OK")


# revision 24
# speedup vs baseline: 238.1484x; 1.1749x over previous
"""Trainium2 Bass kernel for nn_CapsuleLayer (dynamic routing).

Reference computation (B=128, I=1152, P=8, J=10, D=16):
    inputs_hat[b,i,j,d] = sum_p W[i,j,d,p] * inputs[b,i,p]
    b_logits = 0
    3x routing:
        c = softmax_j(b_logits)
        s[b,j,d] = sum_i c[b,i,j] * inputs_hat[b,i,j,d]
        outputs = squash(s)
        b_logits += sum_d inputs_hat[b,i,j,d] * outputs[b,j,d]   (iters 0,1)

Distribution: i-sharded across 8 cores (144 i's per core), full batch B=128
lives in the 128 SBUF partitions on every core.  The only cross-core traffic
is an 80KB AllReduce of the s-partials once per routing iteration.

Per-core layout: everything is [b=128 partitions, free], inputs_hat stored
bf16 as [128, IL, (d,j)=160].  PE materializes inputs_hat via K=32
block-diagonal matmuls (4 i's per matmul, reading the dense k-tiles at
legal 32-aligned stationary bases) and computes iteration-0's s directly
from a K=(i,p) matmul whose AllReduce overlaps the inputs_hat phase.  DVE
runs the bf16 2x elementwise muls and halving-tree reductions; ACT handles
exp and shares PSUM-drain copies; DMA triggering is split across the two
HWDGE queues (sync + scalar) since per-dma_start sequencer cost dominated
the v1 profile (443 DMAs = 300us in the cost model; now ~200 DMAs on 2
queues).
"""

import os
import sys
import functools

import numpy as np

if "/opt/trn_rl_repo" not in sys.path:
    sys.path.insert(0, "/opt/trn_rl_repo")

B = 128
I_FULL = 1152
P_DIM = 8
J = 10
D = 16
JD = D * J  # 160, flattened (d, j): col = d*J + j
NCORES = 8
ROUTINGS = 3
EPS = 1e-7

# lowering knobs (tuned for what actually runs on hardware)
N_HOMES = int(os.environ.get("K_N_HOMES", "3"))   # 32-aligned stationary homes
PSUM_CHUNK = int(os.environ.get("K_PSUM_CHUNK", "1"))  # i's per PSUM bank
K_STOP = int(os.environ.get("K_STOP", "0"))  # debug: truncate after stage N
GPS_FRAC = float(os.environ.get("K_GPS_FRAC", "0"))  # i-frac of muls on gpsimd
BF16_OPS = os.environ.get("K_BF16_OPS", "0") == "1"  # bf16 matmul operands


def build(n_cores, IL, repeat=1):
    """Trace + compile the SPMD Bass program (one program, all cores)."""
    import concourse.bacc as bacc
    import concourse.bass as bass
    import concourse.mybir as mybir
    import concourse.tile as tile
    from concourse.masks import make_identity

    F32 = mybir.dt.float32
    BF16 = mybir.dt.bfloat16
    AF = mybir.ActivationFunctionType
    OP = mybir.AluOpType
    AX = mybir.AxisListType

    assert IL % 16 == 0
    G = IL // 16  # number of 128-row (16 i x 8 p) k-tiles

    nc = bacc.Bacc(
        "TRN2", target_bir_lowering=False, debug=False, num_devices=n_cores
    )
    x_d = nc.dram_tensor("x", [B, IL, P_DIM], F32, kind="ExternalInput").ap()
    w_d = nc.dram_tensor("w", [IL, J, D, P_DIM], F32, kind="ExternalInput").ap()
    out_d = nc.dram_tensor("out", [B, J, D], F32, kind="ExternalOutput").ap()

    with tile.TileContext(nc, num_cores=n_cores) as tc:
        for rep in range(repeat):
            _trace(tc, nc, x_d, w_d, out_d, n_cores, IL, G, F32, BF16, AF,
                   OP, AX, bass, mybir, make_identity, rep)

    nc.compile()
    return nc


def _trace(tc, nc, x_d, w_d, out_d, n_cores, IL, G, F32, BF16, AF, OP, AX,
           bass, mybir, make_identity, rep=0):
    import contextlib

    ctx = contextlib.ExitStack()
    with ctx:
        singles = ctx.enter_context(
            tc.tile_pool(name=f"singles{rep}", bufs=1))
        stage = ctx.enter_context(tc.tile_pool(name=f"stage{rep}", bufs=3))
        big = ctx.enter_context(tc.tile_pool(name=f"big{rep}", bufs=1))
        small = ctx.enter_context(tc.tile_pool(name=f"small{rep}", bufs=3))
        psT = ctx.enter_context(
            tc.tile_pool(name=f"psT{rep}", bufs=2, space="PSUM"))
        psS = ctx.enter_context(
            tc.tile_pool(name=f"psS{rep}", bufs=1, space="PSUM"))
        psIH = ctx.enter_context(
            tc.tile_pool(name=f"psIH{rep}", bufs=4, space="PSUM"))
        dram = ctx.enter_context(
            tc.tile_pool(name=f"dram{rep}", bufs=1, space="DRAM"))

        # ---- constants -------------------------------------------------
        ident = singles.tile([128, 128], F32)
        make_identity(nc, ident[:])
        dummy = singles.tile([128, 1], F32)
        nc.vector.memset(dummy[:], 0.0)
        eps_t = singles.tile([128, 1], F32)
        nc.vector.memset(eps_t[:], EPS)
        # preload ACT tables (Exp / Log) before the hot loop
        nc.scalar.activation(dummy[:], dummy[:], AF.Exp)

        # ---- load inputs, build transposed operands --------------------
        # x_nat: [b, (i p)] fp32
        x_nat = big.tile([128, IL * P_DIM], F32)
        nc.sync.dma_start(out=x_nat[:], in_=x_d.rearrange("b i p -> b (i p)"))

        OPDT = BF16 if BF16_OPS else F32
        # xT[k, g, b]: k-tile g holds rows (i_loc*8+p) for i in [16g,16g+16)
        xT = big.tile([128, G, 128], OPDT)
        for g in range(G):
            pst = psT.tile([128, 128], F32, tag="pst")
            nc.tensor.transpose(
                pst[:], x_nat[:, g * 128:(g + 1) * 128], ident[:])
            nc.vector.tensor_copy(xT[:, g, :], pst[:])

        # W2[k, g, (d j)]: same k-row ordering, free dim is (d,j) = d*J + j.
        # Staging uses (j,d) rows so each j is ONE contiguous-ish DMA (16
        # rows); the PSUM->SBUF copy permutes cols back to (d,j).  DMA
        # triggering alternates sync/tensor queues to parallelize the
        # per-dma_start sequencer cost.
        W2 = big.tile([128, G, JD], OPDT)
        dma_engs = [nc.sync, nc.scalar]
        for g in range(G):
            wna = stage.tile([128, 128], F32, tag="wna")  # rows j*16+d, j<8
            wnb = stage.tile([32, 128], F32, tag="wnb")   # rows (j-8)*16+d
            i0 = 16 * g
            for j in range(J):
                tt, tr = (wna, j * 16) if j < 8 else (wnb, (j - 8) * 16)
                sl = w_d[i0:i0 + 16, j:j + 1, :, :]
                dma_engs[(g * J + j) % 2].dma_start(
                    out=tt[tr:tr + 16, :].rearrange(
                        "r (i p) -> r i p", p=P_DIM),
                    in_=sl.rearrange("i j d p -> (j d) i p"),
                )
            W2g = W2[:, g, :].rearrange("k (d j) -> k d j", d=D, j=J)
            psa = psT.tile([128, 128], F32, tag="pst")
            nc.tensor.transpose(psa[:], wna[:], ident[:])
            nc.vector.tensor_copy(
                W2g[:, :, 0:8],
                psa[:].rearrange("k (j d) -> k j d", j=8, d=D).transpose(
                    [0, 2, 1]))
            psb = psT.tile([128, 32], F32, tag="pst")
            nc.tensor.transpose(psb[:], wnb[:], ident[0:32, 0:32])
            nc.vector.tensor_copy(
                W2g[:, :, 8:10],
                psb[:].rearrange("k (j d) -> k j d", j=2, d=D).transpose(
                    [0, 2, 1]))

        # ---- block-diagonal weight tiles for the inputs_hat matmuls -----
        # K=32 slices of the dense k-tiles are legal stationary bases
        # (0/32/64/96 with explicit tile_position).  Each 32-row group
        # holds 4 i's; the moving operand is a [32, 640] block-diagonal
        # expansion of W2 so the 4 i's don't mix.  Built with
        # partition-preserving on-chip copies (rows 32a+8t == 8*i_loc),
        # no DMA involved.
        w28bd = big.tile([128, G, 4 * JD], F32)
        nc.gpsimd.memset(w28bd[:], 0.0)
        idx = 0
        for g in range(G):
            for a in range(4):
                for t in range(4):
                    r0 = 32 * a + 8 * t
                    dst = w28bd[r0:r0 + 8, g, JD * t:JD * (t + 1)]
                    src = W2[r0:r0 + 8, g, :]
                    if t == 0:
                        # 32-aligned start partition: engine copy is legal
                        nc.vector.tensor_copy(dst, src)
                    else:
                        # sub-32 partition starts: only DMA can address
                        dma_engs[idx % 2].dma_start(out=dst, in_=src)
                        idx += 1

        # ---- iteration-0 s directly from PE (c == 1/J), AllReduce now --
        # s0T[(d j), b] = sum_{(i,p)} W2[k, dj] * xT[k, b]
        ps_a = psS.tile([128, 128], F32, tag="s0a")
        ps_b = psS.tile([32, 128], F32, tag="s0b")
        for g in range(G):
            nc.tensor.matmul(ps_a[:], W2[:, g, 0:128], xT[:, g, :],
                             start=(g == 0), stop=(g == G - 1))
        for g in range(G):
            nc.tensor.matmul(ps_b[:], W2[:, g, 128:JD], xT[:, g, :],
                             start=(g == 0), stop=(g == G - 1))
        # scale by 1/J while copying out of PSUM, then transpose to [b, dj]
        s0T_a = stage.tile([128, 128], F32, tag="s0Ta")
        s0T_b = stage.tile([32, 128], F32, tag="s0Tb")
        nc.scalar.mul(s0T_a[:], ps_a[:], 1.0 / J)
        nc.scalar.mul(s0T_b[:], ps_b[:], 1.0 / J)
        s0p = small.tile([128, JD], F32, tag="spart")
        pst = psT.tile([128, 128], F32, tag="pst")
        nc.tensor.transpose(pst[:], s0T_a[:], ident[:])
        nc.vector.tensor_copy(s0p[:, 0:128], pst[:])
        pstb2 = psT.tile([128, 32], F32, tag="pst")
        nc.tensor.transpose(pstb2[:], s0T_b[:], ident[0:32, 0:32])
        nc.vector.tensor_copy(s0p[:, 128:JD], pstb2[:])

        def all_reduce(s_part, tag):
            cc_in = dram.tile([B, JD], F32, name=f"ccin_{tag}")
            cc_out = dram.tile([B, JD], F32, name=f"ccout_{tag}",
                               addr_space="Shared")
            nc.gpsimd.dma_start(out=cc_in[:], in_=s_part[:])
            if n_cores > 1 and os.environ.get("K_NO_CC", "0") != "1":
                nc.gpsimd.collective_compute(
                    "AllReduce",
                    OP.add,
                    replica_groups=[list(range(n_cores))],
                    ins=[cc_in[:].opt()],
                    outs=[cc_out[:].opt()],
                )
            else:
                nc.gpsimd.dma_start(out=cc_out[:], in_=cc_in[:])
            s_glob = small.tile([128, JD], F32, tag="sglob")
            nc.gpsimd.dma_start(out=s_glob[:], in_=cc_out[:])
            return s_glob

        s0g = all_reduce(s0p, "s0")  # overlaps the IH phase below

        # ---- materialize inputs_hat: IH[b, i, (d j)] bf16 --------------
        IH = big.tile([128, IL, JD], BF16)
        kk = 0
        for g in range(G):
            for a in range(4):
                for h in range(2):
                    i0 = 16 * g + 4 * a + 2 * h
                    if i0 >= IL:
                        continue
                    ps = psIH.tile([128, 2 * JD], F32, tag="ih")
                    nc.tensor.matmul(
                        ps[:], xT[32 * a:32 * a + 32, g, :],
                        w28bd[32 * a:32 * a + 32, g,
                              2 * JD * h:2 * JD * (h + 1)],
                        start=True, stop=True, tile_position=(32 * a, 0))
                    dst = IH[:, i0:i0 + 2, :]
                    if kk % 3 == 0:
                        nc.vector.tensor_copy(dst, ps[:])
                    else:
                        nc.scalar.copy(dst, ps[:])
                    kk += 1

        # ---- routing helpers -------------------------------------------
        XB = big.tile([128, IL, JD], BF16)  # scratch for muls + trees
        L = big.tile([128, IL, J], F32)     # routing logits

        def squash(s_glob, want_bf16):
            """squash along d of s_glob[128,(d j)] -> (f32, bf16|None)."""
            sq = small.tile([128, JD], F32, tag="sq")
            nc.vector.tensor_mul(sq[:], s_glob[:], s_glob[:])
            s2 = small.tile([128, J], F32, tag="s2")
            nc.vector.reduce_sum(
                s2[:], sq.rearrange("b (d j) -> b j d", d=D, j=J), axis=AX.X)
            # t = sqrt(s2 + eps) = exp(0.5 * ln(s2 + eps))
            lt = small.tile([128, J], F32, tag="lt")
            nc.scalar.activation(lt[:], s2[:], AF.Ln, bias=eps_t[:])
            rt = small.tile([128, J], F32, tag="rt")
            nc.scalar.activation(rt[:], lt[:], AF.Exp, scale=-0.5)  # 1/sqrt
            u = small.tile([128, J], F32, tag="u")
            nc.vector.tensor_scalar_add(u[:], s2[:], 1.0)
            ru = small.tile([128, J], F32, tag="ru")
            nc.vector.reciprocal(ru[:], u[:])
            sc = small.tile([128, J], F32, tag="sc")
            nc.vector.tensor_mul(sc[:], s2[:], ru[:])
            nc.vector.tensor_mul(sc[:], sc[:], rt[:])
            o_f = small.tile([128, JD], F32, tag="of")
            sc_b = sc[:].unsqueeze(1).broadcast_to([128, D, J])
            nc.vector.tensor_tensor(
                o_f.rearrange("b (d j) -> b d j", d=D, j=J),
                s_glob.rearrange("b (d j) -> b d j", d=D, j=J),
                sc_b, op=OP.mult)
            o_b = None
            if want_bf16:
                o_b = small.tile([128, JD], BF16, tag="ob")
                nc.vector.tensor_copy(o_b[:], o_f[:])
            return o_f, o_b

        def agreement(o_b, first):
            """XB = IH * o_b (bcast over i); tree-reduce d; into L."""
            IS = IL - int(IL * GPS_FRAC) if GPS_FRAC > 0 else IL
            nc.vector.tensor_tensor(
                XB[:, 0:IS, :], IH[:, 0:IS, :],
                o_b[:].unsqueeze(1).broadcast_to([128, IS, JD]), op=OP.mult)
            if IS < IL:
                nc.gpsimd.tensor_tensor(
                    XB[:, IS:IL, :], IH[:, IS:IL, :],
                    o_b[:].unsqueeze(1).broadcast_to([128, IL - IS, JD]),
                    op=OP.mult)
            w = JD
            while w > 2 * J:
                h = w // 2
                nc.vector.tensor_tensor(
                    XB[:, :, 0:h], XB[:, :, 0:h], XB[:, :, h:w], op=OP.add)
                w = h
            # final level: w == 2J -> write/accumulate into logits
            if first:
                nc.vector.tensor_tensor(
                    L[:], XB[:, :, 0:J], XB[:, :, J:2 * J], op=OP.add)
            else:
                a1 = big.tile([128, IL, J], F32, tag="a1")
                nc.vector.tensor_tensor(
                    a1[:], XB[:, :, 0:J], XB[:, :, J:2 * J], op=OP.add)
                nc.vector.tensor_tensor(L[:], L[:], a1[:], op=OP.add)

        def softmax():
            """c = softmax_j(L) -> bf16 [128, IL, J]."""
            E = big.tile([128, IL, J], F32, tag="E")
            nc.scalar.activation(E[:], L[:], AF.Exp)
            Z = small.tile([128, IL], F32, tag="Z")
            nc.vector.reduce_sum(Z[:], E[:], axis=AX.X)
            R = small.tile([128, IL], F32, tag="R")
            nc.vector.reciprocal(R[:], Z[:])
            Cb = big.tile([128, IL, J], BF16, tag="Cb")
            nc.vector.tensor_tensor(
                Cb[:], E[:], R[:].unsqueeze(2).broadcast_to([128, IL, J]),
                op=OP.mult)
            return Cb

        def weighted_sum(Cb, tag):
            """XB = IH * c (bcast over d); tree-reduce i -> s_part."""
            XBv = XB.rearrange("b i (d j) -> b i d j", d=D, j=J)
            IHv = IH.rearrange("b i (d j) -> b i d j", d=D, j=J)
            Cbv = Cb[:].unsqueeze(2).broadcast_to([128, IL, D, J])
            IS = IL - int(IL * GPS_FRAC) if GPS_FRAC > 0 else IL
            nc.vector.tensor_tensor(
                XBv[:, 0:IS], IHv[:, 0:IS], Cbv[:, 0:IS], op=OP.mult)
            if IS < IL:
                nc.gpsimd.tensor_tensor(
                    XBv[:, IS:IL], IHv[:, IS:IL], Cbv[:, IS:IL], op=OP.mult)
            n = IL
            while n > 1:
                h = n // 2
                nc.vector.tensor_tensor(
                    XB[:, 0:h, :], XB[:, 0:h, :], XB[:, h:2 * h, :],
                    op=OP.add)
                if n % 2:
                    nc.vector.tensor_tensor(
                        XB[:, 0:1, :], XB[:, 0:1, :], XB[:, n - 1:n, :],
                        op=OP.add)
                n = h
            s_part = small.tile([128, JD], F32, tag="spart")
            nc.vector.tensor_copy(s_part[:], XB[:, 0, :])
            return s_part

        # ---- routing loop ----------------------------------------------
        # iter 0
        o_f, o_b = squash(s0g, want_bf16=True)
        agreement(o_b, first=True)
        Cb = softmax()
        # iter 1
        s1p = weighted_sum(Cb, "s1")
        s1g = all_reduce(s1p, "s1")
        o_f, o_b = squash(s1g, want_bf16=True)
        agreement(o_b, first=False)
        Cb = softmax()
        # iter 2
        s2p = weighted_sum(Cb, "s2")
        s2g = all_reduce(s2p, "s2")
        o_f, _ = squash(s2g, want_bf16=False)

        # reorder (d,j) -> (j,d) and store
        OUTJD = small.tile([128, J, D], F32, tag="outjd")
        nc.vector.tensor_copy(
            OUTJD[:], o_f.rearrange("b (d j) -> b j d", d=D, j=J))
        nc.sync.dma_start(out=out_d[:], in_=OUTJD[:])


@functools.lru_cache(maxsize=None)
def _get_nc():
    return build(NCORES, I_FULL // NCORES)


def kernel(inputs, W):
    """Full-input entry point: inputs [128,1152,8] f32, W [1,1152,10,16,8]."""
    from concourse.bass_utils import run_bass_kernel_spmd

    inputs = np.ascontiguousarray(np.asarray(inputs), dtype=np.float32)
    W0 = np.ascontiguousarray(np.asarray(W)[0], dtype=np.float32)
    IL = I_FULL // NCORES
    nc = _get_nc()
    in_maps = [
        {
            "x": np.ascontiguousarray(inputs[:, c * IL:(c + 1) * IL, :]),
            "w": np.ascontiguousarray(W0[c * IL:(c + 1) * IL]),
        }
        for c in range(NCORES)
    ]
    res = run_bass_kernel_spmd(nc, in_maps, core_ids=list(range(NCORES)))
    return np.asarray(res.results[0]["out"], dtype=np.float32)


if __name__ == "__main__":
    nc = build(1, 16)
    print("built OK")
